# revision 33
# baseline (speedup 1.0000x reference)
"""GAT kernel for trn2, 8-core SPMD.

v3: single-chunk AllGathers (fewer 15us collective overheads, better modeled
bandwidth tier); indicator matrix `ind` SBUF-resident (loaded once, reused by
both GAT layers); htab scatter declares a 128-row out AP (descriptor count is
derived from the out AP; the full-tensor AP was 20x overcharged); exp values
computed compactly [128,J_W,H] and broadcast inside the msg multiply instead
of materializing [128,J_W,H,D]; msg multiplies batched 8 chunks per op;
layer-2 denominator column fused into the aggregation matmul (psr2 129 cols).
"""
import numpy as np
import ml_dtypes

import concourse.bass as bass
import concourse.bacc as bacc
import concourse.mybir as mybir
import concourse.tile as tile
from concourse.masks import make_identity

dt = mybir.dt
F32 = dt.float32
BF16 = dt.bfloat16
F8 = dt.float8e4
I16 = dt.int16
I32 = dt.int32

SENT_NEG = -1.0e30
J_W = 16           # chunks per window
W_E = J_W * 128    # edges per window


class GatConfig:
    def __init__(self, n_nodes=20000, n_graphs=128, n_cores=8,
                 d_in=256, h1=8, d1=64, d2=128, fp8=True, ag_chunks=1):
        self.fp8 = fp8
        self.ag_chunks = ag_chunks
        fb = 1 if fp8 else 2            # feature bytes in gather rows
        self.n_nodes = n_nodes
        self.n_graphs = n_graphs
        self.n_cores = n_cores
        self.d_in = d_in
        self.h1 = h1
        self.d1 = d1
        self.hd1 = h1 * d1        # 512
        self.d2 = d2              # 128
        assert n_nodes % n_cores == 0
        self.slice = n_nodes // n_cores
        self.slice_pad = (self.slice + 1 + 127) // 128 * 128
        self.trows = n_cores * self.slice_pad
        self.n_tiles = self.slice_pad // 128
        r1 = self.hd1 * fb + 2 * h1 * 4
        self.row1b = (r1 + 255) // 256 * 256     # 768 fp8 / 1280 bf16
        self.row1_el = self.row1b // fb          # row len in feat elems
        self.row1_f32 = self.row1b // 4
        self.el1_f32 = self.hd1 * fb // 4        # f32 col where el starts
        r2 = d2 * fb + 2 * 4
        self.row2b = (r2 + 255) // 256 * 256     # 256 fp8 / 512 bf16
        self.row2_el = self.row2b // fb
        self.row2_f32 = self.row2b // 4
        self.el2_f32 = d2 * fb // 4
        self.sent_row = self.slice
        assert self.el1_f32 + 2 * h1 <= self.row1_f32
        assert self.el2_f32 + 2 <= self.row2_f32


def build_host_data(cfg, x, W1, a_l1, a_r1, b1, W2, a_l2, a_r2, b2,
                    edge_src, edge_dst, graph_ids):
    c = cfg
    x = np.asarray(x, np.float32)
    W1 = np.asarray(W1, np.float32); W2 = np.asarray(W2, np.float32)
    a_l1 = np.asarray(a_l1, np.float32); a_r1 = np.asarray(a_r1, np.float32)
    a_l2 = np.asarray(a_l2, np.float32); a_r2 = np.asarray(a_r2, np.float32)
    src = np.asarray(edge_src).astype(np.int64)
    dst = np.asarray(edge_dst).astype(np.int64)
    gid = np.asarray(graph_ids).astype(np.int64)

    w_el1 = np.stack([W1[:, h * c.d1:(h + 1) * c.d1] @ a_l1[h] for h in range(c.h1)], 1)
    w_er1 = np.stack([W1[:, h * c.d1:(h + 1) * c.d1] @ a_r1[h] for h in range(c.h1)], 1)
    W1p = np.concatenate([W1, w_el1, w_er1], axis=1)
    W2p = np.concatenate([W2, W2 @ a_l2[0][:, None], W2 @ a_r2[0][:, None]], axis=1)

    perm = np.argsort(dst, kind="stable")
    src_s = src[perm]; dst_s = dst[perm]
    deg = np.bincount(dst_s, minlength=c.n_nodes)

    core_windows = []
    n_win = 0
    for cc in range(c.n_cores):
        nlo, nhi = cc * c.slice, (cc + 1) * c.slice
        wins = []
        n0 = nlo
        while n0 < nhi:
            n1 = n0
            ecount = 0
            while n1 < nhi and (n1 - n0) < 128 and ecount + deg[n1] <= W_E:
                ecount += deg[n1]
                n1 += 1
            assert n1 > n0, f"node {n0} degree {deg[n0]} > {W_E}"
            wins.append((n0, n1))
            n0 = n1
        core_windows.append(wins)
        n_win = max(n_win, len(wins))
    epad = n_win * W_E
    jtot = epad // 128

    edge_starts = np.searchsorted(dst_s, np.arange(c.n_nodes + 1))
    eye128 = np.eye(128, dtype=np.float32)

    per_core = []
    n0s_all = []
    for cc in range(c.n_cores):
        wins = core_windows[cc]
        hrows = c.slice_pad // c.ag_chunks
        sent_g = ((c.sent_row // hrows) * c.n_cores * hrows + 0 * hrows
                  + c.sent_row % hrows)
        src_g = np.full(epad, sent_g, np.int64)
        dstrel = np.zeros(epad, np.int64)
        hidx = np.full((128, n_win), c.slice_pad, np.int64)
        gidw = np.full((128, n_win), c.n_graphs, np.int64)
        hrows2 = c.slice_pad // 4
        sent2_g = ((c.sent_row // hrows2) * c.n_cores * hrows2 + 0 * hrows2
                   + c.sent_row % hrows2)
        src2_g = np.full(epad, sent2_g, np.int64)
        n0s = []
        for w, (n0, n1) in enumerate(wins):
            e0, e1 = edge_starts[n0], edge_starts[n1]
            ne = e1 - e0
            base = w * W_E
            es = src_s[e0:e1]; ed = dst_s[e0:e1]
            ec, er_ = es // c.slice, es % c.slice
            src_g[base:base + ne] = ((er_ // hrows) * c.n_cores * hrows
                                     + ec * hrows + er_ % hrows)
            src2_g[base:base + ne] = ((er_ // hrows2) * c.n_cores * hrows2
                                      + ec * hrows2 + er_ % hrows2)
            dstrel[base:base + ne] = ed - n0
            nw = n1 - n0
            hidx[:nw, w] = (n0 - cc * c.slice) + np.arange(nw)
            gidw[:nw, w] = gid[n0:n1]
            n0s.append(n0 - cc * c.slice)
        while len(n0s) < n_win:
            n0s.append(c.slice_pad)
        n0s_all.append(n0s)
        assert src_g.max() < 32768 and src2_g.max() < 32768
        src_i16 = np.tile(src_g.astype(np.int16).reshape(epad // 16, 16).T, (8, 1)).copy()
        src2_i16 = np.tile(src2_g.astype(np.int16).reshape(epad // 16, 16).T, (8, 1)).copy()

        # indicator matrices, host-built:
        # ind[e_pos, w, ci, n] = 1 iff edge (w,ci,e_pos) has dstrel == n
        oh = eye128[dstrel]                               # [epad, 128]
        oh4 = oh.reshape(n_win, J_W, 128, 128)            # [w, ci, e, n]
        indb = np.ascontiguousarray(
            oh4.transpose(2, 0, 1, 3)).astype(ml_dtypes.bfloat16)   # [e,w,ci,n]
        indTb = np.ascontiguousarray(
            oh4.transpose(3, 0, 1, 2)).astype(ml_dtypes.bfloat16)   # [n,w,ci,e]
        # gind[n_pos, w, g] = 1 iff node at window-pos n_pos has graph id g
        ghot = np.zeros((128, n_win, c.n_graphs), np.float32)
        valid = gidw < c.n_graphs
        pp, ww = np.nonzero(valid)
        ghot[pp, ww, gidw[pp, ww]] = 1.0
        gindb = np.ascontiguousarray(ghot).astype(ml_dtypes.bfloat16)

        nlo = cc * c.slice
        xT = np.zeros((c.d_in, c.slice_pad), ml_dtypes.bfloat16)
        xT[:, :c.slice] = x[nlo:nlo + c.slice].T.astype(ml_dtypes.bfloat16)

        tidx = (np.arange(c.n_tiles)[None, :] * 128
                + np.arange(128)[:, None]).astype(np.int32)
        per_core.append({
            "tidx": np.ascontiguousarray(tidx),
            "xT": xT, "w1p": W1p.astype(ml_dtypes.bfloat16),
            "w2p": W2p.astype(ml_dtypes.bfloat16),
            "b1t": np.tile(np.asarray(b1, np.float32)[None, :], (128, 1)),
            "b2t": np.tile(np.asarray(b2, np.float32)[None, :], (128, 1)),
            "srcidx": src_i16,
            "srcidx2": src2_i16,
            "hidx": np.ascontiguousarray(hidx.astype(np.int32)),
            "indb": indb.reshape(128, n_win * J_W * 128),
            "indTb": indTb.reshape(128, n_win * J_W * 128),
            "gindb": gindb.reshape(128, n_win * c.n_graphs),
        })

    counts = np.bincount(gid, minlength=c.n_graphs).astype(np.float32)
    has_b = (bool(np.any(np.asarray(b1))), bool(np.any(np.asarray(b2))))
    wneed = []
    for t in range(c.n_tiles):
        hi = min(128 * (t + 1), c.slice)
        w_t = 0
        for cc in range(c.n_cores):
            ends = [n1 - cc * c.slice for (_n0, n1) in core_windows[cc]]
            w_cc = next(w for w, e in enumerate(ends) if e >= hi)
            w_t = max(w_t, w_cc)
        wneed.append(w_t)
    return per_core, counts, n_win, has_b, n0s_all, tuple(wneed)



def _gather2(nc, out_tile, in_ap, idx, wcol, n, elem, step=None):
    """Two 1024-idx dma_gathers (SWDGE ring caps one instruction at 1024 descs)."""
    half = n // 2
    for g in range(2):
        nc.gpsimd.dma_gather(
            out_ap=out_tile[:, g * (half // 128):(g + 1) * (half // 128)],
            in_ap=in_ap,
            idxs_ap=idx[:, wcol + g * (half // 16):wcol + (g + 1) * (half // 16)],
            num_idxs=half, num_idxs_reg=half, elem_size=elem,
            **({"elem_step": step} if step is not None else {}))


def build_program(cfg, n_win, sim1=False, fake_ag=False, has_b=(False, False),
                  repeat=1, shared=True, phases='ABCD', n0s=None,
                  gat_bufs=2, no_ag=False, wneed=None):
    fake_ag = fake_ag or sim1
    c = cfg
    epad = n_win * W_E
    jtot = epad // 128
    nc = bacc.Bacc("TRN2", target_bir_lowering=False, debug=False,
                   num_devices=1 if sim1 else c.n_cores)

    t_xT = nc.dram_tensor("xT", [c.d_in, c.slice_pad], BF16, kind="ExternalInput")
    t_w1p = nc.dram_tensor("w1p", [c.d_in, c.hd1 + 2 * c.h1], BF16, kind="ExternalInput")
    t_w2p = nc.dram_tensor("w2p", [c.hd1, c.d2 + 2], BF16, kind="ExternalInput")
    t_srcidx = nc.dram_tensor("srcidx", [128, epad // 16], I16, kind="ExternalInput")
    t_srcidx2 = nc.dram_tensor("srcidx2", [128, epad // 16], I16, kind="ExternalInput")
    t_hidx = nc.dram_tensor("hidx", [128, n_win], I32, kind="ExternalInput")
    t_tidx = nc.dram_tensor("tidx", [128, c.n_tiles], I32, kind="ExternalInput")
    t_indb = nc.dram_tensor("indb", [128, n_win * J_W * 128], BF16, kind="ExternalInput")
    t_indTb = nc.dram_tensor("indTb", [128, n_win * J_W * 128], BF16, kind="ExternalInput")
    t_gindb = nc.dram_tensor("gindb", [128, n_win * 128], BF16, kind="ExternalInput")
    t_b1t = nc.dram_tensor("b1t", [128, c.hd1], F32, kind="ExternalInput")
    t_b2t = nc.dram_tensor("b2t", [128, c.d2], F32, kind="ExternalInput")
    t_pool = nc.dram_tensor("pool", [128, c.d2], F32, kind="ExternalOutput")

    FD = F8 if c.fp8 else BF16
    addr_sp = "Shared" if shared else "Local"
    bounce1 = nc.dram_tensor("bounce1", [c.slice_pad + 128, c.row1_el], FD)
    table1 = nc.dram_tensor("table1", [c.trows, c.row1_el], FD, addr_space=addr_sp)
    bounce2 = nc.dram_tensor("bounce2", [c.slice_pad + 128, c.row2_el], FD)
    table2 = nc.dram_tensor("table2", [c.trows, c.row2_el], FD, addr_space=addr_sp)
    htab = nc.dram_tensor("htab", [c.slice_pad + 136, c.hd1], BF16)

    K1 = c.d_in // 128
    K2 = c.hd1 // 128
    NW1 = c.hd1 + 2 * c.h1
    NW2 = c.d2 + 2
    h1, d1, hd1, d2 = c.h1, c.d1, c.hd1, c.d2

    with tile.TileContext(nc) as tc:
        with (
            tc.tile_pool(name="res", bufs=1) as res,
            tc.tile_pool(name="wk", bufs=4) as wk,
            tc.tile_pool(name="gat", bufs=gat_bufs) as gat,
            tc.tile_pool(name="chunk", bufs=2) as chk,
            tc.tile_pool(name="ps", bufs=3, space="PSUM") as ps,
            tc.tile_pool(name="pss", bufs=2, space="PSUM") as pss,
            tc.tile_pool(name="pser", bufs=2, space="PSUM") as pser,
            tc.tile_pool(name="pspool", bufs=1, space="PSUM") as pspool,
        ):
            srcidx = res.tile([128, epad // 16], I16)
            srcidx2 = res.tile([128, epad // 16], I16)
            nc.sync.dma_start(out=srcidx2[:], in_=t_srcidx2[:])
            hidx = res.tile([128, n_win], I32)
            tidx = res.tile([128, c.n_tiles], I32)
            nc.sync.dma_start(out=tidx[:], in_=t_tidx[:])
            gindb = res.tile([128, n_win, 128], BF16)
            nc.sync.dma_start(out=srcidx[:], in_=t_srcidx[:])
            nc.sync.dma_start(out=hidx[:], in_=t_hidx[:])
            nc.sync.dma_start(out=gindb[:], in_=t_gindb[:])
            # resident indicator matrix, shared by both layers (loaded
            # late, see below, so the 10MB transfer overlaps the AllGather)
            indr = res.tile([128, n_win, J_W, 128], BF16)
            b1t = res.tile([128, hd1], F32)
            b2t = res.tile([128, d2], F32)
            if has_b[0]:
                nc.sync.dma_start(out=b1t[:], in_=t_b1t[:])
            if has_b[1]:
                nc.sync.dma_start(out=b2t[:], in_=t_b2t[:])
            ident = res.tile([128, 128], BF16)
            make_identity(nc, ident[:])
            zeros = res.tile([128, hd1], F32)
            nc.vector.memset(zeros[:], 0.0)
            zerosb = res.tile([128, hd1], BF16)
            nc.vector.memset(zerosb[:], 0.0)

            xtall = []
            for k in range(K1):
                t = res.tile([128, c.slice_pad], BF16, tag=f"xtall{k}")
                nc.sync.dma_start(out=t[:], in_=t_xT[k * 128:(k + 1) * 128, :])
                xtall.append(t)
            w1k = []
            for k in range(K1):
                t = res.tile([128, NW1], BF16, tag=f"w1k{k}")
                nc.sync.dma_start(out=t[:], in_=t_w1p[k * 128:(k + 1) * 128, :])
                w1k.append(t)
            w2k = []
            for k in range(K2):
                t = res.tile([128, NW2], BF16, tag=f"w2k{k}")
                nc.sync.dma_start(out=t[:], in_=t_w2p[k * 128:(k + 1) * 128, :])
                w2k.append(t)

            b1f32 = bounce1[:].bitcast(F32)
            t1f32 = table1[:].bitcast(F32)
            b2f32 = bounce2[:].bitcast(F32)
            t2f32 = table2[:].bitcast(F32)

            # zero the extended tail rows once (er-window loads may read them)
            nc.sync.dma_start(out=b1f32[c.slice_pad:c.slice_pad + 128, 0:c.row1_f32],
                              in_=zeros[:, 0:c.row1_f32])
            nc.sync.dma_start(out=b2f32[c.slice_pad:c.slice_pad + 128, 0:c.row2_f32],
                              in_=zeros[:, 0:c.row2_f32])

            def load_erw(out_t, base_f32ap, col0, ncols, w):
                """er values for window w's nodes: rows hidx[:,w] of local bounce."""
                if sim1:
                    n0 = n0s[w]
                    nc.sync.dma_start(out=out_t,
                                      in_=base_f32ap[n0:n0 + 128, col0:col0 + ncols])
                else:
                    nc.gpsimd.indirect_dma_start(
                        out=out_t,
                        out_offset=None,
                        in_=base_f32ap,
                        in_offset=bass.IndirectOffsetOnAxis(ap=hidx[:, w:w + 1], axis=0),
                        element_offset=col0)

            def emit_atile(t):
                psf = ps.tile([128, hd1], F32, space="PSUM", tag="big")
                pse = pss.tile([128, 2 * h1], F32, space="PSUM", tag="small")
                for k in range(K1):
                    nc.tensor.matmul(out=psf[:],
                                     lhsT=xtall[k][:, t * 128:(t + 1) * 128],
                                     rhs=w1k[k][:, 0:hd1],
                                     start=(k == 0), stop=(k == K1 - 1))
                for k in range(K1):
                    nc.tensor.matmul(out=pse[:],
                                     lhsT=xtall[k][:, t * 128:(t + 1) * 128],
                                     rhs=w1k[k][:, hd1:NW1],
                                     start=(k == 0), stop=(k == K1 - 1))
                fbe = wk.tile([128, c.el1_f32 + 2 * h1], F32, tag="fbe")
                nc.vector.tensor_copy(out=fbe[:].bitcast(FD)[:, 0:hd1], in_=psf[:])
                nc.vector.tensor_copy(out=fbe[:, c.el1_f32:c.el1_f32 + 2 * h1],
                                      in_=pse[:])
                r0, r1 = t * 128, (t + 1) * 128
                nc.sync.dma_start(out=b1f32[r0:r1, 0:c.el1_f32 + 2 * h1],
                                  in_=fbe[:])

            def emit_a_tail(first):
                sent = wk.tile([1, 2 * h1], F32, tag="sent")
                nc.vector.memset(sent[:], SENT_NEG)
                nc.sync.dma_start(
                    out=b1f32[c.sent_row:c.sent_row + 1, c.el1_f32:c.el1_f32 + 2 * h1],
                    in_=sent[:])
                hrows = c.slice_pad // c.ag_chunks
                gh = c.n_cores * hrows
                for hh in range(c.ag_chunks):
                    r0, r1 = hh * hrows, (hh + 1) * hrows
                    if no_ag:
                        pass
                    elif fake_ag:
                        for cc in range(c.n_cores):
                            nc.sync.dma_start(
                                out=table1[hh * gh + cc * hrows:hh * gh + (cc + 1) * hrows, :],
                                in_=bounce1[r0:r1, :])
                    else:
                        nc.gpsimd.collective_compute(
                            "AllGather", mybir.AluOpType.bypass,
                            replica_groups=[list(range(c.n_cores))],
                            ins=[bounce1[r0:r1, :]], outs=[table1[hh * gh:(hh + 1) * gh, :]])
                if first:
                    nc.sync.dma_start(out=indr[:], in_=t_indb[:])
                if c.slice_pad > c.slice:
                    npad = c.slice_pad - c.slice
                    nc.sync.dma_start(out=htab[c.slice:c.slice_pad, :],
                                      in_=zerosb[:npad, :])

            for _rep in range(repeat):
                if 'A' in phases and _rep == 0:
                    for t in range(c.n_tiles):
                        emit_atile(t)
                    emit_a_tail(True)

                if 'B' in phases:
                    # ---- Phase B: layer-1 edges (C tiles interleaved) ----
                    def emit_window(w):
                        featg = gat.tile([128, J_W, c.row1_el], FD, tag="featg")
                        _gather2(nc, featg, table1[:], srcidx, w * 128, W_E, c.row1_el)
                        indTt = gat.tile([128, J_W, 128], BF16, tag="indTt")
                        nc.sync.dma_start(
                            out=indTt[:],
                            in_=t_indTb[:, w * J_W * 128:(w + 1) * J_W * 128])
                        erw = wk.tile([128, h1], F32, tag="erw")
                        load_erw(erw[:], b1f32[:], c.el1_f32 + h1, h1, w)
                        erwb = wk.tile([128, h1], BF16, tag="erwb")
                        nc.vector.tensor_copy(out=erwb[:], in_=erw[:])
                        erps = pser.tile([128, J_W, h1], F32, space="PSUM", tag="erps")
                        for ci in range(J_W):
                            nc.tensor.matmul(out=erps[:, ci], lhsT=indTt[:, ci],
                                             rhs=erwb[:], start=True, stop=True)

                        fg32 = featg[:].bitcast(F32)
                        elv = fg32[:, :, c.el1_f32:c.el1_f32 + h1]
                        s = wk.tile([128, J_W, h1], F32, tag="s1")
                        nc.vector.tensor_tensor(out=s[:], in0=elv, in1=erps[:],
                                                op=mybir.AluOpType.add)
                        slr = wk.tile([128, J_W, h1], F32, tag="slr1")
                        nc.vector.tensor_scalar_mul(slr[:], s[:], 0.2)
                        slr2 = wk.tile([128, J_W, h1], F32, tag="slr1b")
                        nc.vector.tensor_tensor(out=slr2[:], in0=slr[:], in1=s[:],
                                                op=mybir.AluOpType.max)
                        exps = wk.tile([128, J_W, h1], BF16, tag="exps")
                        nc.scalar.activation(out=exps[:], in_=slr2[:],
                                             func=mybir.ActivationFunctionType.Exp)

                        psr = ps.tile([128, hd1], F32, space="PSUM", tag="big")
                        psd = pss.tile([128, h1], F32, space="PSUM", tag="small")
                        for hf in range(2):
                            ci0 = hf * (J_W // 2)
                            msg = chk.tile([128, J_W // 2, hd1], BF16, tag="msg")
                            nc.vector.tensor_tensor(
                                out=msg[:], in0=featg[:, ci0:ci0 + J_W // 2, 0:hd1],
                                in1=exps[:, ci0:ci0 + J_W // 2].to_broadcast(
                                    [128, J_W // 2, h1, d1]),
                                op=mybir.AluOpType.mult)
                            for j in range(J_W // 2):
                                ci = ci0 + j
                                nc.tensor.matmul(out=psr[:], lhsT=indr[:, w, ci], rhs=msg[:, j],
                                                 start=(ci == 0), stop=(ci == J_W - 1))
                                nc.tensor.matmul(out=psd[:], lhsT=indr[:, w, ci],
                                                 rhs=exps[:, ci],
                                                 start=(ci == 0), stop=(ci == J_W - 1))
                        dg = wk.tile([128, h1], F32, tag="dg1")
                        nc.vector.tensor_scalar_max(dg[:], psd[:], 1e-30)
                        rec = wk.tile([128, h1], F32, tag="rec1")
                        nc.vector.reciprocal(out=rec[:], in_=dg[:])
                        hwin = wk.tile([128, hd1], BF16, tag="hwin")
                        nc.vector.tensor_tensor(out=hwin[:], in0=psr[:],
                                                in1=rec[:].to_broadcast([128, h1, d1]),
                                                op=mybir.AluOpType.mult)
                        if has_b[0]:
                            nc.vector.tensor_tensor(out=hwin[:], in0=hwin[:], in1=b1t[:],
                                                    op=mybir.AluOpType.add)
                        if sim1:
                            n0 = n0s[w]
                            nc.gpsimd.dma_start(out=htab[n0:n0 + 128, :], in_=hwin[:])
                        else:
                            # 128-row out AP: descriptors are counted from the
                            # out AP; indices address rows of the full tensor.
                            nc.gpsimd.indirect_dma_start(
                                out=htab[0:128, :],
                                out_offset=bass.IndirectOffsetOnAxis(ap=hidx[:, w:w + 1], axis=0),
                                in_=hwin[:], in_offset=None)

                    def emit_ctile(t):
                        ht = wk.tile([128, hd1], BF16, tag="ht")
                        if sim1:
                            nc.sync.dma_start(out=ht[:],
                                              in_=htab[t * 128:(t + 1) * 128, :])
                        else:
                            nc.gpsimd.indirect_dma_start(
                                out=ht[:], out_offset=None, in_=htab[:],
                                in_offset=bass.IndirectOffsetOnAxis(
                                    ap=tidx[:, t:t + 1], axis=0))
                        hTs = []
                        for k in range(K2):
                            pst = ps.tile([128, 128], BF16, space="PSUM", tag="big")
                            nc.tensor.transpose(out=pst[:], in_=ht[:, k * 128:(k + 1) * 128],
                                                identity=ident[:])
                            hT = wk.tile([128, 128], BF16, tag=f"hT{k}")
                            nc.vector.tensor_copy(out=hT[:], in_=pst[:])
                            hTs.append(hT)
                        psf2 = ps.tile([128, NW2], F32, space="PSUM", tag="big")
                        for k in range(K2):
                            nc.tensor.matmul(out=psf2[:], lhsT=hTs[k][:], rhs=w2k[k][:],
                                             start=(k == 0), stop=(k == K2 - 1))
                        f2bf = wk.tile([128, d2], FD, tag="f2bf")
                        nc.vector.tensor_copy(out=f2bf[:], in_=psf2[:, 0:d2])
                        elr2 = wk.tile([128, 2], F32, tag="elr2")
                        nc.vector.tensor_copy(out=elr2[:], in_=psf2[:, d2:d2 + 2])
                        r0, r1 = t * 128, (t + 1) * 128
                        nc.sync.dma_start(out=bounce2[r0:r1, 0:d2], in_=f2bf[:])
                        nc.sync.dma_start(out=b2f32[r0:r1, c.el2_f32:c.el2_f32 + 2],
                                          in_=elr2[:])

                    hrows2 = c.slice_pad // 4
                    gh2 = c.n_cores * hrows2

                    def emit_ag2(hh):
                        r0, r1 = hh * hrows2, (hh + 1) * hrows2
                        if no_ag:
                            pass
                        elif fake_ag:
                            for cc in range(c.n_cores):
                                nc.sync.dma_start(
                                    out=table2[hh * gh2 + cc * hrows2:hh * gh2 + (cc + 1) * hrows2, :],
                                    in_=bounce2[r0:r1, :])
                        else:
                            nc.gpsimd.collective_compute(
                                "AllGather", mybir.AluOpType.bypass,
                                replica_groups=[list(range(c.n_cores))],
                                ins=[bounce2[r0:r1, :]], outs=[table2[hh * gh2:(hh + 1) * gh2, :]])

                    # interleave: emit each C tile as soon as the windows
                    # covering its htab rows are out; AG2 chunks 0-2 fire as
                    # their quarter of C completes, overlapping the tail of B,
                    # so only the last chunk sits on the critical path.
                    do_c = 'C' in phases
                    qt = c.n_tiles // 4
                    t_next = 0
                    for w in range(n_win):
                        emit_window(w)
                        while (do_c and t_next < c.n_tiles
                               and wneed[t_next] <= w):
                            emit_ctile(t_next)
                            t_next += 1
                            if t_next % qt == 0 and t_next // qt <= 3:
                                emit_ag2(t_next // qt - 1)
                    if do_c:
                        while t_next < c.n_tiles:
                            emit_ctile(t_next)
                            t_next += 1
                            if t_next % qt == 0 and t_next // qt <= 3:
                                emit_ag2(t_next // qt - 1)
                        sent2 = wk.tile([1, 2], F32, tag="sent2")
                        nc.vector.memset(sent2[:], SENT_NEG)
                        nc.sync.dma_start(
                            out=b2f32[c.sent_row:c.sent_row + 1, c.el2_f32:c.el2_f32 + 2],
                            in_=sent2[:])
                        # next iteration's phase A: its PE/DVE work fills the
                        # collective wait, and its AllGather goes on the queue
                        # BEFORE AG2 chunk 1, so this iteration's phase D (and
                        # AG2c1) runs concurrently with the next B phase.
                        if 'A' in phases and _rep + 1 < repeat:
                            for ta in range(c.n_tiles):
                                emit_atile(ta)
                            emit_a_tail(False)
                            emit_ag2(3)
                        else:
                            emit_ag2(3)

                if 'D' in phases:
                    # ---- Phase D: layer-2 edges + pooling; next iteration's
                    # phase-A tiles interleaved so its AllGather can launch
                    # while D is still draining (software pipeline).
                    pspl = pspool.tile([128, d2], F32, space="PSUM", tag="pspl")

                    def emit_dwindow(w):
                        f2g = gat.tile([128, J_W, c.row2_el], FD, tag="f2g")
                        _gather2(nc, f2g, table2[:], srcidx2, w * 128, W_E, c.row2_el)
                        indTt = gat.tile([128, J_W, 128], BF16, tag="indTt")
                        nc.sync.dma_start(
                            out=indTt[:],
                            in_=t_indTb[:, w * J_W * 128:(w + 1) * J_W * 128])
                        erw = wk.tile([128, 1], F32, tag="er2w")
                        load_erw(erw[:], b2f32[:], c.el2_f32 + 1, 1, w)
                        erwb = wk.tile([128, 1], BF16, tag="er2wb")
                        nc.vector.tensor_copy(out=erwb[:], in_=erw[:])
                        erps = pser.tile([128, J_W, h1], F32, space="PSUM", tag="erps")
                        for ci in range(J_W):
                            nc.tensor.matmul(out=erps[:, ci, 0:1], lhsT=indTt[:, ci],
                                             rhs=erwb[:], start=True, stop=True)

                        f2g32 = f2g[:].bitcast(F32)
                        el2v = f2g32[:, :, c.el2_f32:c.el2_f32 + 1]
                        s = wk.tile([128, J_W, 1], F32, tag="s2")
                        nc.vector.tensor_tensor(out=s[:], in0=el2v, in1=erps[:, :, 0:1],
                                                op=mybir.AluOpType.add)
                        slr = wk.tile([128, J_W, 1], F32, tag="slr2_")
                        nc.vector.tensor_scalar_mul(slr[:], s[:], 0.2)
                        slr2 = wk.tile([128, J_W, 1], F32, tag="slr2b")
                        nc.vector.tensor_tensor(out=slr2[:], in0=slr[:], in1=s[:],
                                                op=mybir.AluOpType.max)
                        exps2 = wk.tile([128, J_W, 1], BF16, tag="exps2")
                        nc.scalar.activation(out=exps2[:], in_=slr2[:],
                                             func=mybir.ActivationFunctionType.Exp)

                        # msg2e: cols 0:d2 = feat*alpha_exp, col d2 = alpha_exp
                        # so the denominator falls out of the same aggregation.
                        msg2e = chk.tile([128, J_W, d2 + 1], BF16, tag="msg2e")
                        nc.vector.tensor_tensor(
                            out=msg2e[:, :, 0:d2], in0=f2g[:, :, 0:d2],
                            in1=exps2[:].to_broadcast([128, J_W, 1, d2]),
                            op=mybir.AluOpType.mult)
                        nc.vector.tensor_copy(out=msg2e[:, :, d2:d2 + 1], in_=exps2[:])

                        psr2 = ps.tile([128, d2 + 1], F32, space="PSUM", tag="big")
                        for ci in range(J_W):
                            nc.tensor.matmul(out=psr2[:], lhsT=indr[:, w, ci],
                                             rhs=msg2e[:, ci],
                                             start=(ci == 0), stop=(ci == J_W - 1))
                        dg = wk.tile([128, 1], F32, tag="dg2")
                        nc.vector.tensor_scalar_max(dg[:], psr2[:, d2:d2 + 1], 1e-30)
                        rec = wk.tile([128, 1], F32, tag="rec2")
                        nc.vector.reciprocal(out=rec[:], in_=dg[:])
                        h2w = wk.tile([128, d2], BF16, tag="h2w")
                        nc.vector.tensor_tensor(out=h2w[:], in0=psr2[:, 0:d2],
                                                in1=rec[:].to_broadcast([128, d2]),
                                                op=mybir.AluOpType.mult)
                        if has_b[1]:
                            nc.vector.tensor_tensor(out=h2w[:], in0=h2w[:], in1=b2t[:],
                                                    op=mybir.AluOpType.add)
                        nc.tensor.matmul(out=pspl[:], lhsT=gindb[:, w], rhs=h2w[:],
                                         start=(w == 0), stop=(w == n_win - 1))

                    for w in range(n_win):
                        emit_dwindow(w)
                    poolsb = res.tile([128, d2], F32)
                    nc.vector.tensor_copy(out=poolsb[:], in_=pspl[:])
                    nc.sync.dma_start(out=t_pool[:], in_=poolsb[:])

    nc.compile()
    return nc


def host_reduce(cfg, results, counts):
    pool_sum = np.zeros((128, cfg.d2), np.float32)
    for cc in range(cfg.n_cores):
        pool_sum += np.asarray(results[cc]["pool"], np.float32)
    hg = pool_sum[:cfg.n_graphs] / np.maximum(counts, 1.0)[:, None]
    return hg.reshape(cfg.n_graphs, 1, cfg.d2).astype(np.float32)




_PROG_CACHE = {}


def kernel(**inputs):
    """GAT forward on 8 trn2 NeuronCores. Full inputs in, [128,1,128] f32 out."""
    import concourse.bass_utils as bass_utils
    cfg = GatConfig()
    per_core, counts, n_win, has_b, _n0s, wneed = build_host_data(cfg, **inputs)
    key = (n_win, has_b, wneed)
    nc = _PROG_CACHE.get(key)
    if nc is None:
        nc = build_program(cfg, n_win, has_b=has_b, wneed=wneed)
        _PROG_CACHE[key] = nc
    res = bass_utils.run_bass_kernel_spmd(nc, per_core,
                                          core_ids=list(range(cfg.n_cores)))
    return host_reduce(cfg, res.results, counts)


# revision 35
# speedup vs baseline: 1.0116x; 1.0116x over previous
"""GAT kernel for trn2, 8-core SPMD.

v3: single-chunk AllGathers (fewer 15us collective overheads, better modeled
bandwidth tier); indicator matrix `ind` SBUF-resident (loaded once, reused by
both GAT layers); htab scatter declares a 128-row out AP (descriptor count is
derived from the out AP; the full-tensor AP was 20x overcharged); exp values
computed compactly [128,J_W,H] and broadcast inside the msg multiply instead
of materializing [128,J_W,H,D]; msg multiplies batched 8 chunks per op;
layer-2 denominator column fused into the aggregation matmul (psr2 129 cols).
"""
import numpy as np
import ml_dtypes

import concourse.bass as bass
import concourse.bacc as bacc
import concourse.mybir as mybir
import concourse.tile as tile
from concourse.masks import make_identity

dt = mybir.dt
F32 = dt.float32
BF16 = dt.bfloat16
F8 = dt.float8e4
I16 = dt.int16
I32 = dt.int32

SENT_NEG = -1.0e30
J_W = 16           # chunks per window
W_E = J_W * 128    # edges per window


class GatConfig:
    def __init__(self, n_nodes=20000, n_graphs=128, n_cores=8,
                 d_in=256, h1=8, d1=64, d2=128, fp8=True, ag_chunks=1):
        self.fp8 = fp8
        self.ag_chunks = ag_chunks
        fb = 1 if fp8 else 2            # feature bytes in gather rows
        self.n_nodes = n_nodes
        self.n_graphs = n_graphs
        self.n_cores = n_cores
        self.d_in = d_in
        self.h1 = h1
        self.d1 = d1
        self.hd1 = h1 * d1        # 512
        self.d2 = d2              # 128
        assert n_nodes % n_cores == 0
        self.slice = n_nodes // n_cores
        self.slice_pad = (self.slice + 1 + 127) // 128 * 128
        self.trows = n_cores * self.slice_pad
        self.n_tiles = self.slice_pad // 128
        r1 = self.hd1 * fb + 2 * h1 * 4
        self.row1b = (r1 + 255) // 256 * 256     # 768 fp8 / 1280 bf16
        self.row1_el = self.row1b // fb          # row len in feat elems
        self.row1_f32 = self.row1b // 4
        self.el1_f32 = self.hd1 * fb // 4        # f32 col where el starts
        r2 = d2 * fb + 2 * 4
        self.row2b = (r2 + 255) // 256 * 256     # 256 fp8 / 512 bf16
        self.row2_el = self.row2b // fb
        self.row2_f32 = self.row2b // 4
        self.el2_f32 = d2 * fb // 4
        self.sent_row = self.slice
        assert self.el1_f32 + 2 * h1 <= self.row1_f32
        assert self.el2_f32 + 2 <= self.row2_f32


def build_host_data(cfg, x, W1, a_l1, a_r1, b1, W2, a_l2, a_r2, b2,
                    edge_src, edge_dst, graph_ids):
    c = cfg
    x = np.asarray(x, np.float32)
    W1 = np.asarray(W1, np.float32); W2 = np.asarray(W2, np.float32)
    a_l1 = np.asarray(a_l1, np.float32); a_r1 = np.asarray(a_r1, np.float32)
    a_l2 = np.asarray(a_l2, np.float32); a_r2 = np.asarray(a_r2, np.float32)
    src = np.asarray(edge_src).astype(np.int64)
    dst = np.asarray(edge_dst).astype(np.int64)
    gid = np.asarray(graph_ids).astype(np.int64)

    w_el1 = np.stack([W1[:, h * c.d1:(h + 1) * c.d1] @ a_l1[h] for h in range(c.h1)], 1)
    w_er1 = np.stack([W1[:, h * c.d1:(h + 1) * c.d1] @ a_r1[h] for h in range(c.h1)], 1)
    W1p = np.concatenate([W1, w_el1, w_er1], axis=1)
    W2p = np.concatenate([W2, W2 @ a_l2[0][:, None], W2 @ a_r2[0][:, None]], axis=1)

    perm = np.argsort(dst, kind="stable")
    src_s = src[perm]; dst_s = dst[perm]
    deg = np.bincount(dst_s, minlength=c.n_nodes)

    core_windows = []
    n_win = 0
    for cc in range(c.n_cores):
        nlo, nhi = cc * c.slice, (cc + 1) * c.slice
        wins = []
        n0 = nlo
        while n0 < nhi:
            n1 = n0
            ecount = 0
            while n1 < nhi and (n1 - n0) < 128 and ecount + deg[n1] <= W_E:
                ecount += deg[n1]
                n1 += 1
            assert n1 > n0, f"node {n0} degree {deg[n0]} > {W_E}"
            wins.append((n0, n1))
            n0 = n1
        core_windows.append(wins)
        n_win = max(n_win, len(wins))
    epad = n_win * W_E
    jtot = epad // 128

    edge_starts = np.searchsorted(dst_s, np.arange(c.n_nodes + 1))
    eye128 = np.eye(128, dtype=np.float32)

    per_core = []
    n0s_all = []
    for cc in range(c.n_cores):
        wins = core_windows[cc]
        hrows = c.slice_pad // c.ag_chunks
        sent_g = ((c.sent_row // hrows) * c.n_cores * hrows + 0 * hrows
                  + c.sent_row % hrows)
        src_g = np.full(epad, sent_g, np.int64)
        dstrel = np.zeros(epad, np.int64)
        hidx = np.full((128, n_win), c.slice_pad, np.int64)
        gidw = np.full((128, n_win), c.n_graphs, np.int64)
        hrows2 = c.slice_pad // 5
        sent2_g = ((c.sent_row // hrows2) * c.n_cores * hrows2 + 0 * hrows2
                   + c.sent_row % hrows2)
        src2_g = np.full(epad, sent2_g, np.int64)
        n0s = []
        for w, (n0, n1) in enumerate(wins):
            e0, e1 = edge_starts[n0], edge_starts[n1]
            ne = e1 - e0
            base = w * W_E
            es = src_s[e0:e1]; ed = dst_s[e0:e1]
            ec, er_ = es // c.slice, es % c.slice
            src_g[base:base + ne] = ((er_ // hrows) * c.n_cores * hrows
                                     + ec * hrows + er_ % hrows)
            src2_g[base:base + ne] = ((er_ // hrows2) * c.n_cores * hrows2
                                      + ec * hrows2 + er_ % hrows2)
            dstrel[base:base + ne] = ed - n0
            nw = n1 - n0
            hidx[:nw, w] = (n0 - cc * c.slice) + np.arange(nw)
            gidw[:nw, w] = gid[n0:n1]
            n0s.append(n0 - cc * c.slice)
        while len(n0s) < n_win:
            n0s.append(c.slice_pad)
        n0s_all.append(n0s)
        assert src_g.max() < 32768 and src2_g.max() < 32768
        src_i16 = np.tile(src_g.astype(np.int16).reshape(epad // 16, 16).T, (8, 1)).copy()
        src2_i16 = np.tile(src2_g.astype(np.int16).reshape(epad // 16, 16).T, (8, 1)).copy()

        # indicator matrices, host-built:
        # ind[e_pos, w, ci, n] = 1 iff edge (w,ci,e_pos) has dstrel == n
        oh = eye128[dstrel]                               # [epad, 128]
        oh4 = oh.reshape(n_win, J_W, 128, 128)            # [w, ci, e, n]
        indb = np.ascontiguousarray(
            oh4.transpose(2, 0, 1, 3)).astype(ml_dtypes.bfloat16)   # [e,w,ci,n]
        indTb = np.ascontiguousarray(
            oh4.transpose(3, 0, 1, 2)).astype(ml_dtypes.bfloat16)   # [n,w,ci,e]
        # gind[n_pos, w, g] = 1 iff node at window-pos n_pos has graph id g
        ghot = np.zeros((128, n_win, c.n_graphs), np.float32)
        valid = gidw < c.n_graphs
        pp, ww = np.nonzero(valid)
        ghot[pp, ww, gidw[pp, ww]] = 1.0
        gindb = np.ascontiguousarray(ghot).astype(ml_dtypes.bfloat16)

        nlo = cc * c.slice
        xT = np.zeros((c.d_in, c.slice_pad), ml_dtypes.bfloat16)
        xT[:, :c.slice] = x[nlo:nlo + c.slice].T.astype(ml_dtypes.bfloat16)

        tidx = (np.arange(c.n_tiles)[None, :] * 128
                + np.arange(128)[:, None]).astype(np.int32)
        per_core.append({
            "tidx": np.ascontiguousarray(tidx),
            "xT": xT, "w1p": W1p.astype(ml_dtypes.bfloat16),
            "w2p": W2p.astype(ml_dtypes.bfloat16),
            "b1t": np.tile(np.asarray(b1, np.float32)[None, :], (128, 1)),
            "b2t": np.tile(np.asarray(b2, np.float32)[None, :], (128, 1)),
            "srcidx": src_i16,
            "srcidx2": src2_i16,
            "hidx": np.ascontiguousarray(hidx.astype(np.int32)),
            "indb": indb.reshape(128, n_win * J_W * 128),
            "indTb": indTb.reshape(128, n_win * J_W * 128),
            "gindb": gindb.reshape(128, n_win * c.n_graphs),
        })

    counts = np.bincount(gid, minlength=c.n_graphs).astype(np.float32)
    has_b = (bool(np.any(np.asarray(b1))), bool(np.any(np.asarray(b2))))
    wneed = []
    for t in range(c.n_tiles):
        hi = min(128 * (t + 1), c.slice)
        w_t = 0
        for cc in range(c.n_cores):
            ends = [n1 - cc * c.slice for (_n0, n1) in core_windows[cc]]
            w_cc = next(w for w, e in enumerate(ends) if e >= hi)
            w_t = max(w_t, w_cc)
        wneed.append(w_t)
    return per_core, counts, n_win, has_b, n0s_all, tuple(wneed)



def _gather2(nc, out_tile, in_ap, idx, wcol, n, elem, step=None):
    """Two 1024-idx dma_gathers (SWDGE ring caps one instruction at 1024 descs)."""
    half = n // 2
    for g in range(2):
        nc.gpsimd.dma_gather(
            out_ap=out_tile[:, g * (half // 128):(g + 1) * (half // 128)],
            in_ap=in_ap,
            idxs_ap=idx[:, wcol + g * (half // 16):wcol + (g + 1) * (half // 16)],
            num_idxs=half, num_idxs_reg=half, elem_size=elem,
            **({"elem_step": step} if step is not None else {}))


def build_program(cfg, n_win, sim1=False, fake_ag=False, has_b=(False, False),
                  repeat=1, shared=True, phases='ABCD', n0s=None,
                  gat_bufs=2, no_ag=False, wneed=None):
    fake_ag = fake_ag or sim1
    c = cfg
    epad = n_win * W_E
    jtot = epad // 128
    nc = bacc.Bacc("TRN2", target_bir_lowering=False, debug=False,
                   num_devices=1 if sim1 else c.n_cores)

    t_xT = nc.dram_tensor("xT", [c.d_in, c.slice_pad], BF16, kind="ExternalInput")
    t_w1p = nc.dram_tensor("w1p", [c.d_in, c.hd1 + 2 * c.h1], BF16, kind="ExternalInput")
    t_w2p = nc.dram_tensor("w2p", [c.hd1, c.d2 + 2], BF16, kind="ExternalInput")
    t_srcidx = nc.dram_tensor("srcidx", [128, epad // 16], I16, kind="ExternalInput")
    t_srcidx2 = nc.dram_tensor("srcidx2", [128, epad // 16], I16, kind="ExternalInput")
    t_hidx = nc.dram_tensor("hidx", [128, n_win], I32, kind="ExternalInput")
    t_tidx = nc.dram_tensor("tidx", [128, c.n_tiles], I32, kind="ExternalInput")
    t_indb = nc.dram_tensor("indb", [128, n_win * J_W * 128], BF16, kind="ExternalInput")
    t_indTb = nc.dram_tensor("indTb", [128, n_win * J_W * 128], BF16, kind="ExternalInput")
    t_gindb = nc.dram_tensor("gindb", [128, n_win * 128], BF16, kind="ExternalInput")
    t_b1t = nc.dram_tensor("b1t", [128, c.hd1], F32, kind="ExternalInput")
    t_b2t = nc.dram_tensor("b2t", [128, c.d2], F32, kind="ExternalInput")
    t_pool = nc.dram_tensor("pool", [128, c.d2], F32, kind="ExternalOutput")

    FD = F8 if c.fp8 else BF16
    addr_sp = "Shared" if shared else "Local"
    bounce1 = nc.dram_tensor("bounce1", [c.slice_pad + 128, c.row1_el], FD)
    table1 = nc.dram_tensor("table1", [c.trows, c.row1_el], FD, addr_space=addr_sp)
    bounce2 = nc.dram_tensor("bounce2", [c.slice_pad + 128, c.row2_el], FD)
    table2 = nc.dram_tensor("table2", [c.trows, c.row2_el], FD, addr_space=addr_sp)
    htab = nc.dram_tensor("htab", [c.slice_pad + 136, c.hd1], BF16)

    K1 = c.d_in // 128
    K2 = c.hd1 // 128
    NW1 = c.hd1 + 2 * c.h1
    NW2 = c.d2 + 2
    h1, d1, hd1, d2 = c.h1, c.d1, c.hd1, c.d2

    with tile.TileContext(nc) as tc:
        with (
            tc.tile_pool(name="res", bufs=1) as res,
            tc.tile_pool(name="wk", bufs=4) as wk,
            tc.tile_pool(name="gat", bufs=gat_bufs) as gat,
            tc.tile_pool(name="chunk", bufs=2) as chk,
            tc.tile_pool(name="ps", bufs=3, space="PSUM") as ps,
            tc.tile_pool(name="pss", bufs=2, space="PSUM") as pss,
            tc.tile_pool(name="pser", bufs=2, space="PSUM") as pser,
            tc.tile_pool(name="pspool", bufs=1, space="PSUM") as pspool,
        ):
            srcidx = res.tile([128, epad // 16], I16)
            srcidx2 = res.tile([128, epad // 16], I16)
            nc.sync.dma_start(out=srcidx2[:], in_=t_srcidx2[:])
            hidx = res.tile([128, n_win], I32)
            tidx = res.tile([128, c.n_tiles], I32)
            nc.sync.dma_start(out=tidx[:], in_=t_tidx[:])
            gindb = res.tile([128, n_win, 128], BF16)
            nc.sync.dma_start(out=srcidx[:], in_=t_srcidx[:])
            nc.sync.dma_start(out=hidx[:], in_=t_hidx[:])
            nc.sync.dma_start(out=gindb[:], in_=t_gindb[:])
            # resident indicator matrix, shared by both layers (loaded
            # late, see below, so the 10MB transfer overlaps the AllGather)
            indr = res.tile([128, n_win, J_W, 128], BF16)
            b1t = res.tile([128, hd1], F32)
            b2t = res.tile([128, d2], F32)
            if has_b[0]:
                nc.sync.dma_start(out=b1t[:], in_=t_b1t[:])
            if has_b[1]:
                nc.sync.dma_start(out=b2t[:], in_=t_b2t[:])
            ident = res.tile([128, 128], BF16)
            make_identity(nc, ident[:])
            zeros = res.tile([128, hd1], F32)
            nc.vector.memset(zeros[:], 0.0)
            zerosb = res.tile([128, hd1], BF16)
            nc.vector.memset(zerosb[:], 0.0)

            xtall = []
            for k in range(K1):
                t = res.tile([128, c.slice_pad], BF16, tag=f"xtall{k}")
                nc.sync.dma_start(out=t[:], in_=t_xT[k * 128:(k + 1) * 128, :])
                xtall.append(t)
            w1k = []
            for k in range(K1):
                t = res.tile([128, NW1], BF16, tag=f"w1k{k}")
                nc.sync.dma_start(out=t[:], in_=t_w1p[k * 128:(k + 1) * 128, :])
                w1k.append(t)
            w2k = []
            for k in range(K2):
                t = res.tile([128, NW2], BF16, tag=f"w2k{k}")
                nc.sync.dma_start(out=t[:], in_=t_w2p[k * 128:(k + 1) * 128, :])
                w2k.append(t)

            b1f32 = bounce1[:].bitcast(F32)
            t1f32 = table1[:].bitcast(F32)
            b2f32 = bounce2[:].bitcast(F32)
            t2f32 = table2[:].bitcast(F32)

            # zero the extended tail rows once (er-window loads may read them)
            nc.sync.dma_start(out=b1f32[c.slice_pad:c.slice_pad + 128, 0:c.row1_f32],
                              in_=zeros[:, 0:c.row1_f32])
            nc.sync.dma_start(out=b2f32[c.slice_pad:c.slice_pad + 128, 0:c.row2_f32],
                              in_=zeros[:, 0:c.row2_f32])

            def load_erw(out_t, base_f32ap, col0, ncols, w):
                """er values for window w's nodes: rows hidx[:,w] of local bounce."""
                if sim1:
                    n0 = n0s[w]
                    nc.sync.dma_start(out=out_t,
                                      in_=base_f32ap[n0:n0 + 128, col0:col0 + ncols])
                else:
                    nc.gpsimd.indirect_dma_start(
                        out=out_t,
                        out_offset=None,
                        in_=base_f32ap,
                        in_offset=bass.IndirectOffsetOnAxis(ap=hidx[:, w:w + 1], axis=0),
                        element_offset=col0)

            def emit_atile(t):
                psf = ps.tile([128, hd1], F32, space="PSUM", tag="big")
                pse = pss.tile([128, 2 * h1], F32, space="PSUM", tag="small")
                for k in range(K1):
                    nc.tensor.matmul(out=psf[:],
                                     lhsT=xtall[k][:, t * 128:(t + 1) * 128],
                                     rhs=w1k[k][:, 0:hd1],
                                     start=(k == 0), stop=(k == K1 - 1))
                for k in range(K1):
                    nc.tensor.matmul(out=pse[:],
                                     lhsT=xtall[k][:, t * 128:(t + 1) * 128],
                                     rhs=w1k[k][:, hd1:NW1],
                                     start=(k == 0), stop=(k == K1 - 1))
                fbe = wk.tile([128, c.el1_f32 + 2 * h1], F32, tag="fbe")
                nc.vector.tensor_copy(out=fbe[:].bitcast(FD)[:, 0:hd1], in_=psf[:])
                nc.vector.tensor_copy(out=fbe[:, c.el1_f32:c.el1_f32 + 2 * h1],
                                      in_=pse[:])
                r0, r1 = t * 128, (t + 1) * 128
                nc.sync.dma_start(out=b1f32[r0:r1, 0:c.el1_f32 + 2 * h1],
                                  in_=fbe[:])

            def emit_a_tail(first):
                sent = wk.tile([1, 2 * h1], F32, tag="sent")
                nc.vector.memset(sent[:], SENT_NEG)
                nc.sync.dma_start(
                    out=b1f32[c.sent_row:c.sent_row + 1, c.el1_f32:c.el1_f32 + 2 * h1],
                    in_=sent[:])
                hrows = c.slice_pad // c.ag_chunks
                gh = c.n_cores * hrows
                for hh in range(c.ag_chunks):
                    r0, r1 = hh * hrows, (hh + 1) * hrows
                    if no_ag:
                        pass
                    elif fake_ag:
                        for cc in range(c.n_cores):
                            nc.sync.dma_start(
                                out=table1[hh * gh + cc * hrows:hh * gh + (cc + 1) * hrows, :],
                                in_=bounce1[r0:r1, :])
                    else:
                        nc.gpsimd.collective_compute(
                            "AllGather", mybir.AluOpType.bypass,
                            replica_groups=[list(range(c.n_cores))],
                            ins=[bounce1[r0:r1, :]], outs=[table1[hh * gh:(hh + 1) * gh, :]])
                if first:
                    nc.sync.dma_start(out=indr[:], in_=t_indb[:])
                if c.slice_pad > c.slice:
                    npad = c.slice_pad - c.slice
                    nc.sync.dma_start(out=htab[c.slice:c.slice_pad, :],
                                      in_=zerosb[:npad, :])

            for _rep in range(repeat):
                if 'A' in phases and _rep == 0:
                    for t in range(c.n_tiles):
                        emit_atile(t)
                    emit_a_tail(True)

                if 'B' in phases:
                    # ---- Phase B: layer-1 edges (C tiles interleaved) ----
                    def emit_window(w):
                        featg = gat.tile([128, J_W, c.row1_el], FD, tag="featg")
                        _gather2(nc, featg, table1[:], srcidx, w * 128, W_E, c.row1_el)
                        indTt = gat.tile([128, J_W, 128], BF16, tag="indTt")
                        nc.sync.dma_start(
                            out=indTt[:],
                            in_=t_indTb[:, w * J_W * 128:(w + 1) * J_W * 128])
                        erw = wk.tile([128, h1], F32, tag="erw")
                        load_erw(erw[:], b1f32[:], c.el1_f32 + h1, h1, w)
                        erwb = wk.tile([128, h1], BF16, tag="erwb")
                        nc.vector.tensor_copy(out=erwb[:], in_=erw[:])
                        erps = pser.tile([128, J_W, h1], F32, space="PSUM", tag="erps")
                        for ci in range(J_W):
                            nc.tensor.matmul(out=erps[:, ci], lhsT=indTt[:, ci],
                                             rhs=erwb[:], start=True, stop=True)

                        fg32 = featg[:].bitcast(F32)
                        elv = fg32[:, :, c.el1_f32:c.el1_f32 + h1]
                        s = wk.tile([128, J_W, h1], F32, tag="s1")
                        nc.vector.tensor_tensor(out=s[:], in0=elv, in1=erps[:],
                                                op=mybir.AluOpType.add)
                        slr = wk.tile([128, J_W, h1], F32, tag="slr1")
                        nc.vector.tensor_scalar_mul(slr[:], s[:], 0.2)
                        slr2 = wk.tile([128, J_W, h1], F32, tag="slr1b")
                        nc.vector.tensor_tensor(out=slr2[:], in0=slr[:], in1=s[:],
                                                op=mybir.AluOpType.max)
                        exps = wk.tile([128, J_W, h1], BF16, tag="exps")
                        nc.scalar.activation(out=exps[:], in_=slr2[:],
                                             func=mybir.ActivationFunctionType.Exp)

                        psr = ps.tile([128, hd1], F32, space="PSUM", tag="big")
                        psd = pss.tile([128, h1], F32, space="PSUM", tag="small")
                        for hf in range(2):
                            ci0 = hf * (J_W // 2)
                            msg = chk.tile([128, J_W // 2, hd1], BF16, tag="msg")
                            nc.vector.tensor_tensor(
                                out=msg[:], in0=featg[:, ci0:ci0 + J_W // 2, 0:hd1],
                                in1=exps[:, ci0:ci0 + J_W // 2].to_broadcast(
                                    [128, J_W // 2, h1, d1]),
                                op=mybir.AluOpType.mult)
                            for j in range(J_W // 2):
                                ci = ci0 + j
                                nc.tensor.matmul(out=psr[:], lhsT=indr[:, w, ci], rhs=msg[:, j],
                                                 start=(ci == 0), stop=(ci == J_W - 1))
                                nc.tensor.matmul(out=psd[:], lhsT=indr[:, w, ci],
                                                 rhs=exps[:, ci],
                                                 start=(ci == 0), stop=(ci == J_W - 1))
                        dg = wk.tile([128, h1], F32, tag="dg1")
                        nc.vector.tensor_scalar_max(dg[:], psd[:], 1e-30)
                        rec = wk.tile([128, h1], F32, tag="rec1")
                        nc.vector.reciprocal(out=rec[:], in_=dg[:])
                        hwin = wk.tile([128, hd1], BF16, tag="hwin")
                        nc.vector.tensor_tensor(out=hwin[:], in0=psr[:],
                                                in1=rec[:].to_broadcast([128, h1, d1]),
                                                op=mybir.AluOpType.mult)
                        if has_b[0]:
                            nc.vector.tensor_tensor(out=hwin[:], in0=hwin[:], in1=b1t[:],
                                                    op=mybir.AluOpType.add)
                        if sim1:
                            n0 = n0s[w]
                            nc.gpsimd.dma_start(out=htab[n0:n0 + 128, :], in_=hwin[:])
                        else:
                            # 128-row out AP: descriptors are counted from the
                            # out AP; indices address rows of the full tensor.
                            nc.gpsimd.indirect_dma_start(
                                out=htab[0:128, :],
                                out_offset=bass.IndirectOffsetOnAxis(ap=hidx[:, w:w + 1], axis=0),
                                in_=hwin[:], in_offset=None)

                    def emit_ctile(t):
                        ht = wk.tile([128, hd1], BF16, tag="ht")
                        if sim1:
                            nc.sync.dma_start(out=ht[:],
                                              in_=htab[t * 128:(t + 1) * 128, :])
                        else:
                            nc.gpsimd.indirect_dma_start(
                                out=ht[:], out_offset=None, in_=htab[:],
                                in_offset=bass.IndirectOffsetOnAxis(
                                    ap=tidx[:, t:t + 1], axis=0))
                        hTs = []
                        for k in range(K2):
                            pst = ps.tile([128, 128], BF16, space="PSUM", tag="big")
                            nc.tensor.transpose(out=pst[:], in_=ht[:, k * 128:(k + 1) * 128],
                                                identity=ident[:])
                            hT = wk.tile([128, 128], BF16, tag=f"hT{k}")
                            nc.vector.tensor_copy(out=hT[:], in_=pst[:])
                            hTs.append(hT)
                        psf2 = ps.tile([128, NW2], F32, space="PSUM", tag="big")
                        for k in range(K2):
                            nc.tensor.matmul(out=psf2[:], lhsT=hTs[k][:], rhs=w2k[k][:],
                                             start=(k == 0), stop=(k == K2 - 1))
                        f2bf = wk.tile([128, d2], FD, tag="f2bf")
                        nc.vector.tensor_copy(out=f2bf[:], in_=psf2[:, 0:d2])
                        elr2 = wk.tile([128, 2], F32, tag="elr2")
                        nc.vector.tensor_copy(out=elr2[:], in_=psf2[:, d2:d2 + 2])
                        r0, r1 = t * 128, (t + 1) * 128
                        nc.sync.dma_start(out=bounce2[r0:r1, 0:d2], in_=f2bf[:])
                        nc.sync.dma_start(out=b2f32[r0:r1, c.el2_f32:c.el2_f32 + 2],
                                          in_=elr2[:])

                    hrows2 = c.slice_pad // 5
                    gh2 = c.n_cores * hrows2

                    def emit_ag2(hh):
                        r0, r1 = hh * hrows2, (hh + 1) * hrows2
                        if no_ag:
                            pass
                        elif fake_ag:
                            for cc in range(c.n_cores):
                                nc.sync.dma_start(
                                    out=table2[hh * gh2 + cc * hrows2:hh * gh2 + (cc + 1) * hrows2, :],
                                    in_=bounce2[r0:r1, :])
                        else:
                            nc.gpsimd.collective_compute(
                                "AllGather", mybir.AluOpType.bypass,
                                replica_groups=[list(range(c.n_cores))],
                                ins=[bounce2[r0:r1, :]], outs=[table2[hh * gh2:(hh + 1) * gh2, :]])

                    # interleave: emit each C tile as soon as the windows
                    # covering its htab rows are out; AG2 chunks 0-2 fire as
                    # their quarter of C completes, overlapping the tail of B,
                    # so only the last chunk sits on the critical path.
                    do_c = 'C' in phases
                    qt = c.n_tiles // 5
                    t_next = 0
                    for w in range(n_win):
                        emit_window(w)
                        while (do_c and t_next < c.n_tiles
                               and wneed[t_next] <= w):
                            emit_ctile(t_next)
                            t_next += 1
                            if t_next % qt == 0 and t_next // qt <= 4:
                                emit_ag2(t_next // qt - 1)
                    if do_c:
                        while t_next < c.n_tiles:
                            emit_ctile(t_next)
                            t_next += 1
                            if t_next % qt == 0 and t_next // qt <= 4:
                                emit_ag2(t_next // qt - 1)
                        sent2 = wk.tile([1, 2], F32, tag="sent2")
                        nc.vector.memset(sent2[:], SENT_NEG)
                        nc.sync.dma_start(
                            out=b2f32[c.sent_row:c.sent_row + 1, c.el2_f32:c.el2_f32 + 2],
                            in_=sent2[:])
                        # next iteration's phase A: its PE/DVE work fills the
                        # collective wait, and its AllGather goes on the queue
                        # BEFORE AG2 chunk 1, so this iteration's phase D (and
                        # AG2c1) runs concurrently with the next B phase.
                        if 'A' in phases and _rep + 1 < repeat:
                            for ta in range(c.n_tiles):
                                emit_atile(ta)
                            emit_a_tail(False)
                            emit_ag2(4)
                        else:
                            emit_ag2(4)

                if 'D' in phases:
                    # ---- Phase D: layer-2 edges + pooling; next iteration's
                    # phase-A tiles interleaved so its AllGather can launch
                    # while D is still draining (software pipeline).
                    pspl = pspool.tile([128, d2], F32, space="PSUM", tag="pspl")

                    def emit_dwindow(w):
                        f2g = gat.tile([128, J_W, c.row2_el], FD, tag="f2g")
                        _gather2(nc, f2g, table2[:], srcidx2, w * 128, W_E, c.row2_el)
                        indTt = gat.tile([128, J_W, 128], BF16, tag="indTt")
                        nc.sync.dma_start(
                            out=indTt[:],
                            in_=t_indTb[:, w * J_W * 128:(w + 1) * J_W * 128])
                        erw = wk.tile([128, 1], F32, tag="er2w")
                        load_erw(erw[:], b2f32[:], c.el2_f32 + 1, 1, w)
                        erwb = wk.tile([128, 1], BF16, tag="er2wb")
                        nc.vector.tensor_copy(out=erwb[:], in_=erw[:])
                        erps = pser.tile([128, J_W, h1], F32, space="PSUM", tag="erps")
                        for ci in range(J_W):
                            nc.tensor.matmul(out=erps[:, ci, 0:1], lhsT=indTt[:, ci],
                                             rhs=erwb[:], start=True, stop=True)

                        f2g32 = f2g[:].bitcast(F32)
                        el2v = f2g32[:, :, c.el2_f32:c.el2_f32 + 1]
                        s = wk.tile([128, J_W, 1], F32, tag="s2")
                        nc.vector.tensor_tensor(out=s[:], in0=el2v, in1=erps[:, :, 0:1],
                                                op=mybir.AluOpType.add)
                        slr = wk.tile([128, J_W, 1], F32, tag="slr2_")
                        nc.vector.tensor_scalar_mul(slr[:], s[:], 0.2)
                        slr2 = wk.tile([128, J_W, 1], F32, tag="slr2b")
                        nc.vector.tensor_tensor(out=slr2[:], in0=slr[:], in1=s[:],
                                                op=mybir.AluOpType.max)
                        exps2 = wk.tile([128, J_W, 1], BF16, tag="exps2")
                        nc.scalar.activation(out=exps2[:], in_=slr2[:],
                                             func=mybir.ActivationFunctionType.Exp)

                        # msg2e: cols 0:d2 = feat*alpha_exp, col d2 = alpha_exp
                        # so the denominator falls out of the same aggregation.
                        msg2e = chk.tile([128, J_W, d2 + 1], BF16, tag="msg2e")
                        nc.vector.tensor_tensor(
                            out=msg2e[:, :, 0:d2], in0=f2g[:, :, 0:d2],
                            in1=exps2[:].to_broadcast([128, J_W, 1, d2]),
                            op=mybir.AluOpType.mult)
                        nc.vector.tensor_copy(out=msg2e[:, :, d2:d2 + 1], in_=exps2[:])

                        psr2 = ps.tile([128, d2 + 1], F32, space="PSUM", tag="big")
                        for ci in range(J_W):
                            nc.tensor.matmul(out=psr2[:], lhsT=indr[:, w, ci],
                                             rhs=msg2e[:, ci],
                                             start=(ci == 0), stop=(ci == J_W - 1))
                        dg = wk.tile([128, 1], F32, tag="dg2")
                        nc.vector.tensor_scalar_max(dg[:], psr2[:, d2:d2 + 1], 1e-30)
                        rec = wk.tile([128, 1], F32, tag="rec2")
                        nc.vector.reciprocal(out=rec[:], in_=dg[:])
                        h2w = wk.tile([128, d2], BF16, tag="h2w")
                        nc.vector.tensor_tensor(out=h2w[:], in0=psr2[:, 0:d2],
                                                in1=rec[:].to_broadcast([128, d2]),
                                                op=mybir.AluOpType.mult)
                        if has_b[1]:
                            nc.vector.tensor_tensor(out=h2w[:], in0=h2w[:], in1=b2t[:],
                                                    op=mybir.AluOpType.add)
                        nc.tensor.matmul(out=pspl[:], lhsT=gindb[:, w], rhs=h2w[:],
                                         start=(w == 0), stop=(w == n_win - 1))

                    for w in range(n_win):
                        emit_dwindow(w)
                    poolsb = res.tile([128, d2], F32)
                    nc.vector.tensor_copy(out=poolsb[:], in_=pspl[:])
                    nc.sync.dma_start(out=t_pool[:], in_=poolsb[:])

    nc.compile()
    return nc


def host_reduce(cfg, results, counts):
    pool_sum = np.zeros((128, cfg.d2), np.float32)
    for cc in range(cfg.n_cores):
        pool_sum += np.asarray(results[cc]["pool"], np.float32)
    hg = pool_sum[:cfg.n_graphs] / np.maximum(counts, 1.0)[:, None]
    return hg.reshape(cfg.n_graphs, 1, cfg.d2).astype(np.float32)




_PROG_CACHE = {}


def kernel(**inputs):
    """GAT forward on 8 trn2 NeuronCores. Full inputs in, [128,1,128] f32 out."""
    import concourse.bass_utils as bass_utils
    cfg = GatConfig()
    per_core, counts, n_win, has_b, _n0s, wneed = build_host_data(cfg, **inputs)
    key = (n_win, has_b, wneed)
    nc = _PROG_CACHE.get(key)
    if nc is None:
        nc = build_program(cfg, n_win, has_b=has_b, wneed=wneed)
        _PROG_CACHE[key] = nc
    res = bass_utils.run_bass_kernel_spmd(nc, per_core,
                                          core_ids=list(range(cfg.n_cores)))
    return host_reduce(cfg, res.results, counts)
